# revision 8
# baseline (speedup 1.0000x reference)
"""Trainium2 Bass kernel for nn_DeconvBlock (dynamic-weight transposed conv).

Computes, per sample b:
    w_b   = weight + sum_j feature[b,j] * (t_j * m_j)            (weight synthesis)
    out_b = conv_transpose2d(x_b, w_b, stride=2, pad=1, K=4)     (grouped over batch)
    out   = prelu(out_b + bias, a)

Strategy (data-parallel over batch, 8 cores x 2 samples):
  - conv_transpose(stride 2, K=4, P=1) decomposes into 4 output phases
    (py,px) in {0,1}^2; each phase output pixel is a sum of 4 "taps"
    (ky,kx), each tap a 1x1 conv (matmul over CIN=256) of a +-1 shifted x.
  - Weight synthesis is a tiny per-sample affine combination of 5 small
    tensors; it runs on the host during input sharding, so the device
    sees ready-made per-sample weights (1 MB/sample fp16, laid out
    phase-major so the first 256 KB covers the first PSUM tile).
  - Operands are fp16 (PE streams 16-bit moving operands at 1 col/cycle;
    accumulation stays fp32 in PSUM). 512 matmuls/core of [128x128] @
    [128x512] ~= 110us, the PE roofline for the 4.3 GMAC/core workload.
  - The PE HAM clock gate defaults to half clock and un-throttles only
    after ~3.4us of sustained activity, so a run of dummy matmuls on
    scratch SBUF warms the array while the startup DMAs stream in.
  - Epilogue: ScalarE adds bias (Identity activation w/ per-partition
    bias) and downcasts to fp16; VectorE computes prelu(t) = max(t, a*t)
    while interleaving the 4 phases into contiguous fp16 output rows.
    Each row-block flushes as ONE fully-contiguous DMA (4 KB per
    partition) issued from the Vector queue so output traffic does not
    queue behind input streams. The host upcasts to fp32 after gather.
"""

import numpy as np

import concourse.bass as bass
import concourse.mybir as mybir
from concourse import bacc
from concourse import bass_utils
from concourse.tile import TileContext

B, CIN, COUT, H, W, K, S = 16, 256, 128, 64, 64, 4, 2
NCORES = 8
BPC = B // NCORES  # samples per core
P = 128
NCH = CIN // P     # ic chunks of 128
HP = H + 2         # padded x height/width (zero border of 1)
NROW = 8           # output-phase rows per block
NYB = H // NROW    # row blocks per sample
NWARM = 26         # dummy matmuls to warm the PE clock gate during startup

# phase py -> ((ky, sy), ...): contribution x[y'+sy] * w[ky]
_TAPS = {0: ((1, 0), (3, -1)), 1: ((2, 0), (0, 1))}
_PHASES = [(py, px) for py in (0, 1) for px in (0, 1)]

_COMPILED = None


def _build():
    f32 = mybir.dt.float32
    f16 = mybir.dt.float16
    Alu = mybir.AluOpType
    Act = mybir.ActivationFunctionType

    nc = bacc.Bacc(
        "TRN2", target_bir_lowering=False, debug=False, num_devices=NCORES
    )
    x_d = nc.dram_tensor(
        "x_sh", (BPC, P, NCH, HP, HP), f16, kind="ExternalInput"
    ).ap()
    # weights phase-major: (sample, partition, phase, chunk, tap, cout)
    w_d = nc.dram_tensor(
        "wsyn", (BPC, P, 4, NCH, 4, COUT), f16, kind="ExternalInput"
    ).ap()
    ba_d = nc.dram_tensor("bab", (P, 2), f32, kind="ExternalInput").ap()
    out_d = nc.dram_tensor(
        "out_sh", (BPC, COUT, H * S, W * S), f16, kind="ExternalOutput"
    ).ap()

    with TileContext(nc) as tc:
        with (
            tc.tile_pool(name="const", bufs=1) as const_pool,
            tc.tile_pool(name="warm", bufs=1) as warm_pool,
            tc.tile_pool(name="wsyn_pool", bufs=1) as wsyn_pool,
            tc.tile_pool(name="x_pool", bufs=1) as x_pool,
            tc.tile_pool(name="t_pool", bufs=6) as t_pool,
            tc.tile_pool(name="row_pool", bufs=4) as row_pool,
            tc.tile_pool(name="psum", bufs=7, space="PSUM") as psum_pool,
            tc.tile_pool(name="wpsum", bufs=1, space="PSUM") as wpsum_pool,
        ):
            # ---- PE warm-up: dummy matmuls on scratch SBUF while DMAs run.
            # The HAM clock gate needs ~3.4us of sustained PE activity to
            # raise the array clock from 1.2 to 2.4 GHz.
            dw = warm_pool.tile([P, P], f16, name="dw", tag="dw")
            dx = warm_pool.tile([P, P], f16, name="dx", tag="dx")
            wp = wpsum_pool.tile([P, P], f32, name="wp", tag="wp")
            nc.vector.memset(dw[:], 0.0)
            nc.vector.memset(dx[:], 0.0)
            for _ in range(NWARM):
                nc.tensor.matmul(wp[:], dw[:], dx[:], start=True, stop=True)

            ba_t = const_pool.tile([P, 2], f32)
            nc.scalar.dma_start(ba_t[:], ba_d[:])
            # warm the ScalarE activation table (Identity) during startup DMAs
            scratch_t = const_pool.tile([P, 1], f32)
            nc.vector.memset(scratch_t[:], 0.0)
            nc.scalar.activation(scratch_t[:], scratch_t[:], Act.Identity, scale=1.0)

            wsyn = []
            xt = []
            for s in range(BPC):
                w_s = wsyn_pool.tile(
                    [P, 4, NCH, 4, COUT], f16, name=f"wsyn{s}", tag=f"wsyn{s}"
                )
                wsyn.append(w_s)
                x_s = x_pool.tile(
                    [P, NCH, HP, HP], f16, name=f"xpad{s}", tag=f"xpad{s}"
                )
                xt.append(x_s)
            # Startup DMAs across parallel queues: weights on the sync
            # queue, x rows on the gpsimd queue (both FIFO, so order ==
            # priority within a queue, and dispatch overlaps across queues).
            # The first PSUM tile consumes phase-(0,0) weights + x rows 0:10.
            nc.sync.dma_start(wsyn[0][:, 0], w_d[0, :, 0])
            nc.gpsimd.dma_start(xt[0][:, :, 0:10], x_d[0, :, :, 0:10])
            nc.sync.dma_start(wsyn[0][:, 1:4], w_d[0, :, 1:4])
            nc.gpsimd.dma_start(xt[0][:, :, 10:26], x_d[0, :, :, 10:26])
            nc.gpsimd.dma_start(xt[0][:, :, 26:HP], x_d[0, :, :, 26:HP])
            nc.sync.dma_start(wsyn[1][:], w_d[1])
            nc.sync.dma_start(xt[1][:], x_d[1])

            # ---- main conv loop ----
            # Row-blocks of 8; the final sample's last block is split into
            # 4+2+2 rows so the tail's ACT->prelu->DMA chain after the last
            # matmul is short.
            blocks = [(NROW * i, NROW) for i in range(NYB)]
            last_blocks = blocks[:-1] + [
                (NROW * (NYB - 1), 4),
                (NROW * (NYB - 1) + 4, 2),
                (NROW * (NYB - 1) + 6, 2),
            ]
            for s in range(BPC):
                for by0, nr in last_blocks if s == BPC - 1 else blocks:
                    # row_t free layout (y', py, x', px) == out rows
                    # [2*nr, 2*W] for oy in [2*by0, 2*(by0+nr))
                    row_t = row_pool.tile(
                        [P, nr, 2, W, 2], f16, name="row_t", tag="row_t"
                    )
                    for pi, (py, px) in enumerate(_PHASES):
                        ps = psum_pool.tile([P, nr, W], f32, name="ps", tag="ps")
                        k = 0
                        for c in range(NCH):
                            for ti, (ky, sy) in enumerate(_TAPS[py]):
                                for tj, (kx, sx) in enumerate(_TAPS[px]):
                                    lhsT = wsyn[s][:, pi, c, 2 * ti + tj, :]
                                    y0 = 1 + sy + by0
                                    x0 = 1 + sx
                                    rhs = xt[s][:, c, y0 : y0 + nr, x0 : x0 + W]
                                    nc.tensor.matmul(
                                        ps[:],
                                        lhsT,
                                        rhs,
                                        start=(k == 0),
                                        stop=(k == 7),
                                    )
                                    k += 1
                        tt = t_pool.tile([P, nr, W], f16, name="tt", tag="tt")
                        nc.scalar.activation(
                            tt[:], ps[:], Act.Identity, bias=ba_t[:, 0:1], scale=1.0
                        )
                        # prelu(t) = max(t, a*t), interleaved into row_t
                        nc.vector.scalar_tensor_tensor(
                            row_t[:, :, py, :, px],
                            tt[:],
                            ba_t[:, 1:2],
                            tt[:],
                            op0=Alu.mult,
                            op1=Alu.max,
                        )
                    # one fully-contiguous DMA per block (rows 2*by0..2*by0+2nr)
                    last = s == BPC - 1 and by0 + nr == H
                    if last:
                        # final block: flush each py-half as soon as its two
                        # phases are done; the very last half goes on the
                        # idle sync queue to minimize the tail.
                        nc.gpsimd.dma_start(
                            out_d[s, :, 2 * by0 : 2 * (by0 + nr) - 1 : 2, :],
                            row_t[:, :, 0],
                        )
                        nc.sync.dma_start(
                            out_d[s, :, 2 * by0 + 1 : 2 * (by0 + nr) : 2, :],
                            row_t[:, :, 1],
                        )
                    else:
                        nc.gpsimd.dma_start(
                            out_d[s, :, 2 * by0 : 2 * (by0 + nr), :], row_t[:]
                        )

    nc.compile()
    return nc


def _get_compiled():
    global _COMPILED
    if _COMPILED is None:
        _COMPILED = _build()
    return _COMPILED


def _prep_in_maps(inputs):
    x = np.asarray(inputs["x"], dtype=np.float32)
    xp = np.zeros((B, P, NCH, HP, HP), dtype=np.float16)
    xp[:, :, :, 1 : HP - 1, 1 : HP - 1] = x.reshape(B, NCH, P, H, W).transpose(
        0, 2, 1, 3, 4
    )
    feat = np.asarray(inputs["feature"], dtype=np.float32)
    w = np.asarray(inputs["weight"], dtype=np.float32)
    tms = [
        np.asarray(inputs[f"t_{n}"], dtype=np.float32)[0]
        * np.asarray(inputs[f"m_{n}"], dtype=np.float32)[0]
        for n in ("bayer", "quad", "nano", "qxq")
    ]
    # per-sample weight synthesis on host: (B, CIN, COUT, K, K)
    wsyn = w[None] + sum(
        feat[:, j, None, None, None, None] * tms[j][None] for j in range(4)
    )
    wsyn = wsyn.reshape(B, NCH, P, COUT, K, K).astype(np.float16)
    # phase-major device layout: (B, P, phase, NCH, tap, COUT)
    wph = np.empty((B, P, 4, NCH, 4, COUT), dtype=np.float16)
    for pi, (py, px) in enumerate(_PHASES):
        for ti, (ky, _) in enumerate(_TAPS[py]):
            for tj, (kx, _) in enumerate(_TAPS[px]):
                # (B, NCH, P, COUT) -> (B, P, NCH, COUT)
                wph[:, :, pi, :, 2 * ti + tj, :] = wsyn[
                    :, :, :, :, ky, kx
                ].transpose(0, 2, 1, 3)
    bab = np.concatenate(
        [
            np.asarray(inputs["bias"], dtype=np.float32).reshape(P, 1),
            np.broadcast_to(
                np.asarray(inputs["prelu_a"], dtype=np.float32).reshape(1, 1),
                (P, 1),
            ),
        ],
        axis=1,
    )
    bab = np.ascontiguousarray(bab)
    in_maps = []
    for i in range(NCORES):
        sl = slice(i * BPC, (i + 1) * BPC)
        in_maps.append(
            {
                "x_sh": xp[sl],
                "wsyn": wph[sl],
                "bab": bab,
            }
        )
    return in_maps


def kernel(**inputs):
    nc = _get_compiled()
    in_maps = _prep_in_maps(inputs)
    res = bass_utils.run_bass_kernel_spmd(nc, in_maps, core_ids=list(range(NCORES)))
    return np.concatenate(
        [res.results[i]["out_sh"] for i in range(NCORES)], axis=0
    ).astype(np.float32)


# revision 9
# speedup vs baseline: 1.2512x; 1.2512x over previous
"""Trainium2 Bass kernel for nn_DeconvBlock (dynamic-weight transposed conv).

Computes, per sample b:
    w_b   = weight + sum_j feature[b,j] * (t_j * m_j)            (weight synthesis)
    out_b = conv_transpose2d(x_b, w_b, stride=2, pad=1, K=4)     (grouped over batch)
    out   = prelu(out_b + bias, a)

Strategy (data-parallel over batch, 8 cores x 2 samples):
  - conv_transpose(stride 2, K=4, P=1) decomposes into 4 output phases
    (py,px) in {0,1}^2; each phase output pixel is a sum of 4 "taps"
    (ky,kx), each tap a 1x1 conv (matmul over CIN=256) of a +-1 shifted x.
  - Weight synthesis is a tiny per-sample affine combination of 5 small
    tensors; it runs on the host during input sharding, so the device
    sees ready-made per-sample weights (1 MB/sample fp16, laid out
    phase-major so the first 256 KB covers the first PSUM tile).
  - Operands are fp16 (PE streams 16-bit moving operands at 1 col/cycle;
    accumulation stays fp32 in PSUM). 512 matmuls/core of [128x128] @
    [128x512] ~= 110us, the PE roofline for the 4.3 GMAC/core workload.
  - The PE HAM clock gate defaults to half clock and un-throttles only
    after ~3.4us of sustained activity, so a run of dummy matmuls on
    scratch SBUF warms the array while the startup DMAs stream in.
  - Epilogue: ScalarE adds bias (Identity activation w/ per-partition
    bias) and downcasts to fp16; VectorE computes prelu(t) = max(t, a*t)
    while interleaving the 4 phases into contiguous fp16 output rows.
    Each row-block flushes as ONE fully-contiguous DMA (4 KB per
    partition) issued from the Vector queue so output traffic does not
    queue behind input streams. The host upcasts to fp32 after gather.
"""

import numpy as np

import concourse.bass as bass
import concourse.mybir as mybir
from concourse import bacc
from concourse import bass_utils
from concourse.tile import TileContext

B, CIN, COUT, H, W, K, S = 16, 256, 128, 64, 64, 4, 2
NCORES = 8
BPC = B // NCORES  # samples per core
P = 128
NCH = CIN // P     # ic chunks of 128
HP = H + 2         # padded x height/width (zero border of 1)
NROW = 8           # output-phase rows per block
NYB = H // NROW    # row blocks per sample
NWARM = 26         # dummy matmuls to warm the PE clock gate during startup

# phase py -> ((ky, sy), ...): contribution x[y'+sy] * w[ky]
_TAPS = {0: ((1, 0), (3, -1)), 1: ((2, 0), (0, 1))}
_PHASES = [(py, px) for py in (0, 1) for px in (0, 1)]

_COMPILED = None


def _build():
    f32 = mybir.dt.float32
    f16 = mybir.dt.float16
    Alu = mybir.AluOpType
    Act = mybir.ActivationFunctionType

    nc = bacc.Bacc(
        "TRN2", target_bir_lowering=False, debug=False, num_devices=NCORES
    )
    x_d = nc.dram_tensor(
        "x_sh", (BPC, P, NCH, HP, HP), f16, kind="ExternalInput"
    ).ap()
    # weights phase-major: (sample, partition, phase, chunk, tap, cout)
    w_d = nc.dram_tensor(
        "wsyn", (BPC, P, 4, NCH, 4, COUT), f16, kind="ExternalInput"
    ).ap()
    ba_d = nc.dram_tensor("bab", (P, 2), f32, kind="ExternalInput").ap()
    out_d = nc.dram_tensor(
        "out_sh", (BPC, COUT, H * S, W * S), f16, kind="ExternalOutput"
    ).ap()

    with TileContext(nc) as tc:
        with (
            tc.tile_pool(name="const", bufs=1) as const_pool,
            tc.tile_pool(name="warm", bufs=1) as warm_pool,
            tc.tile_pool(name="wsyn_pool", bufs=1) as wsyn_pool,
            tc.tile_pool(name="x_pool", bufs=1) as x_pool,
            tc.tile_pool(name="t_pool", bufs=6) as t_pool,
            tc.tile_pool(name="row_pool", bufs=4) as row_pool,
            tc.tile_pool(name="psum", bufs=7, space="PSUM") as psum_pool,
            tc.tile_pool(name="wpsum", bufs=1, space="PSUM") as wpsum_pool,
        ):
            # ---- PE warm-up: dummy matmuls on scratch SBUF while DMAs run.
            # The HAM clock gate needs ~3.4us of sustained PE activity to
            # raise the array clock from 1.2 to 2.4 GHz.
            dw = warm_pool.tile([P, P], f16, name="dw", tag="dw")
            dx = warm_pool.tile([P, P], f16, name="dx", tag="dx")
            wp = wpsum_pool.tile([P, P], f32, name="wp", tag="wp")
            nc.vector.memset(dw[:], 0.0)
            nc.vector.memset(dx[:], 0.0)
            for _ in range(NWARM):
                nc.tensor.matmul(wp[:], dw[:], dx[:], start=True, stop=True)

            ba_t = const_pool.tile([P, 2], f32)
            nc.scalar.dma_start(ba_t[:], ba_d[:])
            # warm the ScalarE activation table (Identity) during startup DMAs
            scratch_t = const_pool.tile([P, 1], f32)
            nc.vector.memset(scratch_t[:], 0.0)
            nc.scalar.activation(scratch_t[:], scratch_t[:], Act.Identity, scale=1.0)

            wsyn = []
            xt = []
            for s in range(BPC):
                w_s = wsyn_pool.tile(
                    [P, 4, NCH, 4, COUT], f16, name=f"wsyn{s}", tag=f"wsyn{s}"
                )
                wsyn.append(w_s)
                x_s = x_pool.tile(
                    [P, NCH, HP, HP], f16, name=f"xpad{s}", tag=f"xpad{s}"
                )
                xt.append(x_s)
            # Startup DMAs in priority order on the FIFO sync queue.  The
            # first PSUM tile consumes phase-(0,0) weights + x rows 0:10.
            nc.sync.dma_start(wsyn[0][:, 0], w_d[0, :, 0])
            nc.sync.dma_start(xt[0][:, :, 0:10], x_d[0, :, :, 0:10])
            nc.sync.dma_start(wsyn[0][:, 1:4], w_d[0, :, 1:4])
            nc.sync.dma_start(xt[0][:, :, 10:26], x_d[0, :, :, 10:26])
            nc.sync.dma_start(xt[0][:, :, 26:HP], x_d[0, :, :, 26:HP])
            nc.sync.dma_start(wsyn[1][:], w_d[1])
            nc.sync.dma_start(xt[1][:], x_d[1])

            # ---- main conv loop ----
            # Row-blocks of 8; the final sample's last block is split into
            # 4+2+2 rows so the tail's ACT->prelu->DMA chain after the last
            # matmul is short.
            blocks = [(NROW * i, NROW) for i in range(NYB)]
            last_blocks = blocks[:-1] + [
                (NROW * (NYB - 1), 4),
                (NROW * (NYB - 1) + 4, 2),
                (NROW * (NYB - 1) + 6, 2),
            ]
            for s in range(BPC):
                for by0, nr in last_blocks if s == BPC - 1 else blocks:
                    # row_t free layout (y', py, x', px) == out rows
                    # [2*nr, 2*W] for oy in [2*by0, 2*(by0+nr))
                    row_t = row_pool.tile(
                        [P, nr, 2, W, 2], f16, name="row_t", tag="row_t"
                    )
                    for pi, (py, px) in enumerate(_PHASES):
                        ps = psum_pool.tile([P, nr, W], f32, name="ps", tag="ps")
                        k = 0
                        for c in range(NCH):
                            for ti, (ky, sy) in enumerate(_TAPS[py]):
                                for tj, (kx, sx) in enumerate(_TAPS[px]):
                                    lhsT = wsyn[s][:, pi, c, 2 * ti + tj, :]
                                    y0 = 1 + sy + by0
                                    x0 = 1 + sx
                                    rhs = xt[s][:, c, y0 : y0 + nr, x0 : x0 + W]
                                    nc.tensor.matmul(
                                        ps[:],
                                        lhsT,
                                        rhs,
                                        start=(k == 0),
                                        stop=(k == 7),
                                    )
                                    k += 1
                        tt = t_pool.tile([P, nr, W], f16, name="tt", tag="tt")
                        nc.scalar.activation(
                            tt[:], ps[:], Act.Identity, bias=ba_t[:, 0:1], scale=1.0
                        )
                        # prelu(t) = max(t, a*t), interleaved into row_t
                        nc.vector.scalar_tensor_tensor(
                            row_t[:, :, py, :, px],
                            tt[:],
                            ba_t[:, 1:2],
                            tt[:],
                            op0=Alu.mult,
                            op1=Alu.max,
                        )
                    # one fully-contiguous DMA per block (rows 2*by0..2*by0+2nr)
                    last = s == BPC - 1 and by0 + nr == H
                    if last:
                        # final block: flush each py-half as soon as its two
                        # phases are done; the very last half goes on the
                        # idle sync queue to minimize the tail.
                        nc.gpsimd.dma_start(
                            out_d[s, :, 2 * by0 : 2 * (by0 + nr) - 1 : 2, :],
                            row_t[:, :, 0],
                        )
                        nc.sync.dma_start(
                            out_d[s, :, 2 * by0 + 1 : 2 * (by0 + nr) : 2, :],
                            row_t[:, :, 1],
                        )
                    else:
                        nc.gpsimd.dma_start(
                            out_d[s, :, 2 * by0 : 2 * (by0 + nr), :], row_t[:]
                        )

    nc.compile()
    return nc


def _get_compiled():
    global _COMPILED
    if _COMPILED is None:
        _COMPILED = _build()
    return _COMPILED


def _prep_in_maps(inputs):
    x = np.asarray(inputs["x"], dtype=np.float32)
    xp = np.zeros((B, P, NCH, HP, HP), dtype=np.float16)
    xp[:, :, :, 1 : HP - 1, 1 : HP - 1] = x.reshape(B, NCH, P, H, W).transpose(
        0, 2, 1, 3, 4
    )
    feat = np.asarray(inputs["feature"], dtype=np.float32)
    w = np.asarray(inputs["weight"], dtype=np.float32)
    tms = [
        np.asarray(inputs[f"t_{n}"], dtype=np.float32)[0]
        * np.asarray(inputs[f"m_{n}"], dtype=np.float32)[0]
        for n in ("bayer", "quad", "nano", "qxq")
    ]
    # per-sample weight synthesis on host: (B, CIN, COUT, K, K)
    wsyn = w[None] + sum(
        feat[:, j, None, None, None, None] * tms[j][None] for j in range(4)
    )
    wsyn = wsyn.reshape(B, NCH, P, COUT, K, K).astype(np.float16)
    # phase-major device layout: (B, P, phase, NCH, tap, COUT)
    wph = np.empty((B, P, 4, NCH, 4, COUT), dtype=np.float16)
    for pi, (py, px) in enumerate(_PHASES):
        for ti, (ky, _) in enumerate(_TAPS[py]):
            for tj, (kx, _) in enumerate(_TAPS[px]):
                # (B, NCH, P, COUT) -> (B, P, NCH, COUT)
                wph[:, :, pi, :, 2 * ti + tj, :] = wsyn[
                    :, :, :, :, ky, kx
                ].transpose(0, 2, 1, 3)
    bab = np.concatenate(
        [
            np.asarray(inputs["bias"], dtype=np.float32).reshape(P, 1),
            np.broadcast_to(
                np.asarray(inputs["prelu_a"], dtype=np.float32).reshape(1, 1),
                (P, 1),
            ),
        ],
        axis=1,
    )
    bab = np.ascontiguousarray(bab)
    in_maps = []
    for i in range(NCORES):
        sl = slice(i * BPC, (i + 1) * BPC)
        in_maps.append(
            {
                "x_sh": xp[sl],
                "wsyn": wph[sl],
                "bab": bab,
            }
        )
    return in_maps


def kernel(**inputs):
    nc = _get_compiled()
    in_maps = _prep_in_maps(inputs)
    res = bass_utils.run_bass_kernel_spmd(nc, in_maps, core_ids=list(range(NCORES)))
    return np.concatenate(
        [res.results[i]["out_sh"] for i in range(NCORES)], axis=0
    ).astype(np.float32)
